# revision 21
# baseline (speedup 1.0000x reference)
"""GroupLinear (soft MoE routing) Trainium2 Bass kernel.

Computes out[b,o] = sum_j g[b,j] * (x[b,:] @ W[j,:,:])[o] + (g @ bias_p)[b,o]
for B=16384, G=16, DIN=DOUT=512, fp32.

Sharding: data-parallel over batch across 8 NeuronCores (2048 rows/core);
weight + bias replicated.

Per-core schedule (PE-roofline oriented; the PE stream is the critical path
at ~231 ns per 512-row fp32r matmul, and any multi-us PE stall also drops
the HAM clock to 4/8 for tens of us — so every engine that gates the PE
must stay far ahead):
  - W streams on the SP HWDGE queue; x0/g/bias startup loads on the
    Activation queue so W[0] and x0 transfer concurrently. g tiles for a
    whole phase load as ONE batched DMA ([128, 8, 16]).
  - fp32 warmup matmuls cover the framework preamble -> W[0] arrival window
    and start the HAM clock ramp.
  - group-mix accumulation uses the fused VectorE scalar_tensor_tensor:
    acc = y * g[:,j] + acc (one op per (j,tile) instead of scale+add, with
    the j=0 op seeding from the bias matmul: acc = y*g0 + yb). VectorE is
    the only PSUM drain for y (8.5us per 16-group chain vs 15.1us of PE per
    tile); ScalarE only does the small transpose copies, so neither can
    back-pressure the PE's PSUM rotation.
  - phase A (tiles 0-7): group loop outermost, paced by W arrival; x
    transposes (fp32r identity, 1.5 cy/row) + gT + bias matmul emitted
    per-tile inside the j=0 sweep.
  - phase B (tiles 8-15): tile loop outermost (W resident); each tile's
    output DMA streams out as soon as its chain ends - no tail burst.
"""

import numpy as np

import concourse.bass as bass
import concourse.tile as tile
from concourse import bacc, mybir
from concourse.bass_utils import run_bass_kernel_spmd
from concourse.masks import make_identity

B, G, DIN, DOUT = 16384, 16, 512, 512
NCORES = 8
BC = B // NCORES          # rows per core
P = 128                   # partitions
NBT = BC // P             # batch tiles per core (16)
KC = DIN // P             # contraction chunks (4)
PB = 8                    # batch tiles per phase
NPH = NBT // PB           # phases (2)

F32 = mybir.dt.float32
F32R = mybir.dt.float32r
MULT = mybir.AluOpType.mult
ADD = mybir.AluOpType.add


def _emit(nc, tc, out_ap, x_ap, g_ap, w_ap, bias_ap, ctx):
    const_pool = ctx.enter_context(tc.tile_pool(name="const", bufs=1))
    wpool = ctx.enter_context(tc.tile_pool(name="wpool", bufs=1))
    xpool = ctx.enter_context(tc.tile_pool(name="xpool", bufs=6))
    gpool = ctx.enter_context(tc.tile_pool(name="gpool", bufs=2))
    xtpool = ctx.enter_context(tc.tile_pool(name="xtpool", bufs=PB + 1))
    gtpool = ctx.enter_context(tc.tile_pool(name="gtpool", bufs=PB + 1))
    accpool = ctx.enter_context(tc.tile_pool(name="accpool", bufs=PB + 1))
    ybspool = ctx.enter_context(tc.tile_pool(name="ybspool", bufs=3))
    ps_y = ctx.enter_context(tc.tile_pool(name="ps_y", bufs=4, space="PSUM"))
    ps_yb = ctx.enter_context(tc.tile_pool(name="ps_yb", bufs=2, space="PSUM"))
    ps_t = ctx.enter_context(tc.tile_pool(name="ps_t", bufs=2, space="PSUM"))

    # fp32r identity: transpose cost is keyed on the moving operand (the
    # identity); fp32r streams 1.5 cy/row vs 2.0 for fp32. Built as fp32
    # (gpsimd memset/affine_select reject f32r), rounded into an f32r tile
    # via ScalarE copy (satisfies the rounded-to-FP32r BIR check; 0/1 are
    # exact). A plain fp32 identity serves the fp32 g transposes.
    ident32 = const_pool.tile([P, P], F32, name="ident32")
    make_identity(nc, ident32)
    identr = const_pool.tile([P, P], F32R, name="identr")
    nc.scalar.copy(identr[:], ident32[:])

    # startup loads: Activation HWDGE queue (frees SP for the W stream).
    # ScalarE is drain-idle until the first transposes, ~2us after these.
    xpA = xpool.tile([P, DIN], F32R, tag="xt", name="xt0")
    nc.scalar.dma_start(xpA[:], x_ap[0:P, :])
    gA = gpool.tile([P, PB, G], F32, tag="g", name="gA")
    nc.scalar.dma_start(
        gA[:], g_ap[0:PB * P, :].rearrange("(k p) j -> p k j", p=P)
    )
    bias_sb = const_pool.tile([G, DOUT], F32R, name="bias_sb")
    nc.scalar.dma_start(bias_sb[:], bias_ap[:, :])

    # PE warmup: dependency-free matmuls covering preamble -> W[0] arrival,
    # ramping the HAM clock before the real stream begins.
    dum = const_pool.tile([P, DOUT], F32, name="dum")
    nc.gpsimd.memset(dum[:], 1.0)
    for wi in range(3):
        wps = ps_t.tile([P, DOUT], F32, tag="tps", name="wps")
        nc.tensor.matmul(wps[:], dum[:, 0:P], dum[:], start=True, stop=True)

    w_sb = wpool.tile([P, G * KC * DOUT], F32R, name="w_sb")

    def issue_w(j):
        for ic in range(KC):
            nc.sync.dma_start(
                w_sb[:, (j * KC + ic) * DOUT:(j * KC + ic + 1) * DOUT],
                w_ap[j, ic * P:(ic + 1) * P, :],
            )

    def issue_x(bt, eng):
        xt = xpool.tile([P, DIN], F32R, tag="xt", name=f"xt{bt}")
        eng.dma_start(xt[:], x_ap[bt * P:(bt + 1) * P, :])
        return xt

    # DMA issue occupies a sequencer ~610ns per dma_start, so the phase-A
    # x tiles are split across BOTH HWDGE queues to land before their
    # transposes — a late x tile dips PE duty mid-ramp and trips the HAM
    # idle monitor, which halves the PE clock for ~24us.
    xts = {0: xpA}
    issue_w(0)
    for bt in range(1, 4):
        xts[bt] = issue_x(bt, nc.sync)
    for bt in range(4, PB):
        xts[bt] = issue_x(bt, nc.scalar)
    for j in range(1, G):
        issue_w(j)
    for bt in range(PB, NBT):
        xts[bt] = issue_x(bt, nc.sync)
    gB = gpool.tile([P, PB, G], F32, tag="g", name="gB")
    nc.sync.dma_start(
        gB[:], g_ap[PB * P:2 * PB * P, :].rearrange("(k p) j -> p k j", p=P)
    )

    def issue_transpose_x(xt, bt):
        """PE transposes of x (fp32r); PSUM->SBUF copies on ScalarE."""
        xT = xtpool.tile([P, DIN], F32R, tag="xT", name=f"xT{bt}")
        for ic in range(KC):
            tps = ps_t.tile([P, P], F32R, tag="tps", name="tps")
            nc.tensor.transpose(tps[:], xt[:, ic * P:(ic + 1) * P], identr[:])
            nc.scalar.copy(xT[:, ic * P:(ic + 1) * P], tps[:])
        return xT

    def issue_gt(gsrc, bt):
        gps = ps_t.tile([G, P], F32, tag="tps", name="gps")
        nc.tensor.transpose(gps[:], gsrc, ident32[:])
        gT = gtpool.tile([G, P], F32R, tag="gT", name=f"gT{bt}")
        nc.scalar.copy(gT[:], gps[:])
        return gT

    def matmul_y(xT, j):
        y = ps_y.tile([P, DOUT], F32, tag="y", name="y")
        for ic in range(KC):
            nc.tensor.matmul(
                y[:],
                xT[:, ic * P:(ic + 1) * P],
                w_sb[:, (j * KC + ic) * DOUT:(j * KC + ic + 1) * DOUT],
                start=(ic == 0),
                stop=(ic == KC - 1),
            )
        return y

    def fused_step(acc, y, gcol, seed=None):
        # acc = y * g[:,j] + (yb at j=0 else acc): one VectorE op drains the
        # y PSUM bank, applies the group weight, and accumulates.
        nc.vector.scalar_tensor_tensor(
            acc[:], y[:], gcol, (seed if seed is not None else acc)[:], MULT, ADD
        )

    # ---- phase A: tiles 0..7, group loop outermost (paced by W arrival) ----
    # All transposes run as one block BEFORE the sweeps, while the HAM clock
    # is still cold: their dependency gaps are harmless at 4/8, and the
    # sweeps that follow are a dense 100%-duty matmul stream that keeps the
    # released 8/8 clock from re-throttling.
    trs = {}
    gts = {}
    for k in range(PB):
        trs[k] = issue_transpose_x(xts[k], k)
        gts[k] = issue_gt(gA[:, k, :], k)

    accs = {}
    for j in range(G):
        for k in range(PB):
            gcol = gA[:, k, j:j + 1]
            if j == 0:
                yb = ps_yb.tile([P, DOUT], F32, tag="yb", name=f"yb{k}")
                nc.tensor.matmul(yb[:], gts[k][:], bias_sb[:], start=True, stop=True)
                y = matmul_y(trs[k], j)
                # the fused op may read only one PSUM operand; stage the
                # bias term through SBUF on the (otherwise idle) ScalarE
                ybs = ybspool.tile([P, DOUT], F32, tag="ybs", name=f"ybs{k}")
                nc.scalar.copy(ybs[:], yb[:])
                acc = accpool.tile([P, DOUT], F32, tag="acc", name=f"acc{k}")
                accs[k] = acc
                fused_step(acc, y, gcol, seed=ybs)
            else:
                y = matmul_y(trs[k], j)
                fused_step(accs[k], y, gcol)

    for k in range(PB):
        nc.sync.dma_start(out_ap[k * P:(k + 1) * P, :], accs[k][:])

    # ---- phase B: tiles 8..15, tile loop outermost (W fully resident) ----
    for bt in range(PB, NBT):
        k = bt - PB
        xT = issue_transpose_x(xts[bt], bt)
        gT = issue_gt(gB[:, k, :], bt)
        yb = ps_yb.tile([P, DOUT], F32, tag="yb", name=f"yb{bt}")
        nc.tensor.matmul(yb[:], gT[:], bias_sb[:], start=True, stop=True)
        ybs = ybspool.tile([P, DOUT], F32, tag="ybs", name=f"ybs{bt}")
        nc.scalar.copy(ybs[:], yb[:])
        acc = accpool.tile([P, DOUT], F32, tag="acc", name=f"acc{bt}")
        for j in range(G):
            y = matmul_y(xT, j)
            gcol = gB[:, k, j:j + 1]
            fused_step(acc, y, gcol, seed=ybs if j == 0 else None)
        nc.sync.dma_start(out_ap[bt * P:(bt + 1) * P, :], acc[:])


def _build():
    nc = bacc.Bacc("TRN2", target_bir_lowering=False, debug=False)
    # x/weight/bias declared float32r (same 4-byte layout as fp32 on the
    # host) so DMA feeds the FP32r matmuls/transposes with no conversion
    x_ap = nc.dram_tensor("x", [BC, DIN], F32R, kind="ExternalInput").ap()
    g_ap = nc.dram_tensor("g", [BC, G], F32, kind="ExternalInput").ap()
    w_ap = nc.dram_tensor("weight", [G, DIN, DOUT], F32R, kind="ExternalInput").ap()
    bias_ap = nc.dram_tensor("bias_p", [G, DOUT], F32R, kind="ExternalInput").ap()
    out_ap = nc.dram_tensor("out", [BC, DOUT], F32, kind="ExternalOutput").ap()

    from contextlib import ExitStack

    with tile.TileContext(nc) as tc:
        with ExitStack() as ctx:
            _emit(nc, tc, out_ap, x_ap, g_ap, w_ap, bias_ap, ctx)
    nc.compile()
    return nc


_NC = None
last_result = None


def kernel(x, g, weight, bias_p):
    global _NC, last_result
    if _NC is None:
        _NC = _build()

    x = np.ascontiguousarray(np.asarray(x, dtype=np.float32))
    g = np.ascontiguousarray(np.asarray(g, dtype=np.float32))
    weight = np.ascontiguousarray(np.asarray(weight, dtype=np.float32))
    bias_p = np.ascontiguousarray(np.asarray(bias_p, dtype=np.float32))

    in_maps = [
        {
            "x": x[c * BC:(c + 1) * BC],
            "g": g[c * BC:(c + 1) * BC],
            "weight": weight,
            "bias_p": bias_p,
        }
        for c in range(NCORES)
    ]
    res = run_bass_kernel_spmd(_NC, in_maps, core_ids=list(range(NCORES)))
    last_result = res
    return np.concatenate([r["out"] for r in res.results], axis=0)


# revision 24
# speedup vs baseline: 1.0112x; 1.0112x over previous
"""GroupLinear (soft MoE routing) Trainium2 Bass kernel.

Computes out[b,o] = sum_j g[b,j] * (x[b,:] @ W[j,:,:])[o] + (g @ bias_p)[b,o]
for B=16384, G=16, DIN=DOUT=512, fp32.

Sharding: data-parallel over batch across 8 NeuronCores (2048 rows/core);
weight + bias replicated.

Per-core schedule (PE-roofline oriented; the PE stream is the critical path
at ~231 ns per 512-row fp32r matmul, and any multi-us PE stall also drops
the HAM clock to 4/8 for tens of us — so every engine that gates the PE
must stay far ahead):
  - W streams on the SP HWDGE queue; x0/g/bias startup loads on the
    Activation queue so W[0] and x0 transfer concurrently. g tiles for a
    whole phase load as ONE batched DMA ([128, 8, 16]).
  - fp32 warmup matmuls cover the framework preamble -> W[0] arrival window
    and start the HAM clock ramp.
  - group-mix accumulation uses the fused VectorE scalar_tensor_tensor:
    acc = y * g[:,j] + acc (one op per (j,tile) instead of scale+add, with
    the j=0 op seeding from the bias matmul: acc = y*g0 + yb). VectorE is
    the only PSUM drain for y (8.5us per 16-group chain vs 15.1us of PE per
    tile); ScalarE only does the small transpose copies, so neither can
    back-pressure the PE's PSUM rotation.
  - phase A (tiles 0-7): group loop outermost, paced by W arrival; x
    transposes (fp32r identity, 1.5 cy/row) + gT + bias matmul emitted
    per-tile inside the j=0 sweep.
  - phase B (tiles 8-15): tile loop outermost (W resident); each tile's
    output DMA streams out as soon as its chain ends - no tail burst.
"""

import numpy as np

import concourse.bass as bass
import concourse.tile as tile
from concourse import bacc, mybir
from concourse.bass_utils import run_bass_kernel_spmd
from concourse.masks import make_identity

B, G, DIN, DOUT = 16384, 16, 512, 512
NCORES = 8
BC = B // NCORES          # rows per core
P = 128                   # partitions
NBT = BC // P             # batch tiles per core (16)
KC = DIN // P             # contraction chunks (4)
PB = 8                    # batch tiles per phase
NPH = NBT // PB           # phases (2)

F32 = mybir.dt.float32
F32R = mybir.dt.float32r
MULT = mybir.AluOpType.mult
ADD = mybir.AluOpType.add


def _emit(nc, tc, out_ap, x_ap, g_ap, w_ap, bias_ap, ctx):
    const_pool = ctx.enter_context(tc.tile_pool(name="const", bufs=1))
    wpool = ctx.enter_context(tc.tile_pool(name="wpool", bufs=1))
    xpool = ctx.enter_context(tc.tile_pool(name="xpool", bufs=6))
    gpool = ctx.enter_context(tc.tile_pool(name="gpool", bufs=2))
    xtpool = ctx.enter_context(tc.tile_pool(name="xtpool", bufs=PB + 1))
    gtpool = ctx.enter_context(tc.tile_pool(name="gtpool", bufs=PB + 1))
    accpool = ctx.enter_context(tc.tile_pool(name="accpool", bufs=PB + 1))
    ybspool = ctx.enter_context(tc.tile_pool(name="ybspool", bufs=3))
    ps_y = ctx.enter_context(tc.tile_pool(name="ps_y", bufs=4, space="PSUM"))
    ps_yb = ctx.enter_context(tc.tile_pool(name="ps_yb", bufs=1, space="PSUM"))
    ps_t = ctx.enter_context(tc.tile_pool(name="ps_t", bufs=3, space="PSUM"))

    # fp32r identity: transpose cost is keyed on the moving operand (the
    # identity); fp32r streams 1.5 cy/row vs 2.0 for fp32. Built as fp32
    # (gpsimd memset/affine_select reject f32r), rounded into an f32r tile
    # via ScalarE copy (satisfies the rounded-to-FP32r BIR check; 0/1 are
    # exact). A plain fp32 identity serves the fp32 g transposes.
    ident32 = const_pool.tile([P, P], F32, name="ident32")
    make_identity(nc, ident32)
    identr = const_pool.tile([P, P], F32R, name="identr")
    nc.scalar.copy(identr[:], ident32[:])

    # startup loads: Activation HWDGE queue (frees SP for the W stream).
    # ScalarE is drain-idle until the first transposes, ~2us after these.
    xpA = xpool.tile([P, DIN], F32R, tag="xt", name="xt0")
    nc.scalar.dma_start(xpA[:], x_ap[0:P, :])
    gA = gpool.tile([P, PB, G], F32, tag="g", name="gA")
    nc.scalar.dma_start(
        gA[:], g_ap[0:PB * P, :].rearrange("(k p) j -> p k j", p=P)
    )
    bias_sb = const_pool.tile([G, DOUT], F32R, name="bias_sb")
    nc.scalar.dma_start(bias_sb[:], bias_ap[:, :])

    # PE warmup: dependency-free matmuls covering preamble -> W[0] arrival,
    # ramping the HAM clock before the real stream begins.
    dum = const_pool.tile([P, DOUT], F32, name="dum")
    nc.gpsimd.memset(dum[:], 1.0)
    for wi in range(3):
        wps = ps_t.tile([P, DOUT], F32, tag="tps", name="wps")
        nc.tensor.matmul(wps[:], dum[:, 0:P], dum[:], start=True, stop=True)

    w_sb = wpool.tile([P, G * KC * DOUT], F32R, name="w_sb")

    def issue_w(j):
        for ic in range(KC):
            nc.sync.dma_start(
                w_sb[:, (j * KC + ic) * DOUT:(j * KC + ic + 1) * DOUT],
                w_ap[j, ic * P:(ic + 1) * P, :],
            )

    def issue_x(bt, eng):
        xt = xpool.tile([P, DIN], F32R, tag="xt", name=f"xt{bt}")
        eng.dma_start(xt[:], x_ap[bt * P:(bt + 1) * P, :])
        return xt

    # DMA issue occupies a sequencer ~610ns per dma_start, so the phase-A
    # x tiles are spread over THREE issue paths (SP hwdge, Act hwdge for
    # x0 only, GpSimd swdge) to land before their transposes. A late x
    # tile leaves a >750ns PE hole mid-ramp, which trips the HAM idle
    # monitor and halves the PE clock for ~24-34us; ScalarE must also stay
    # issue-free once transpose drains begin (its queue is in-order).
    xts = {0: xpA}
    issue_w(0)
    for bt in range(1, 4):
        xts[bt] = issue_x(bt, nc.sync)
    for bt in range(4, PB):
        xts[bt] = issue_x(bt, nc.gpsimd)
    for j in range(1, G):
        issue_w(j)
    for bt in range(PB, NBT):
        xts[bt] = issue_x(bt, nc.sync)
    gB = gpool.tile([P, PB, G], F32, tag="g", name="gB")
    nc.sync.dma_start(
        gB[:], g_ap[PB * P:2 * PB * P, :].rearrange("(k p) j -> p k j", p=P)
    )

    def issue_transpose_x(xt, bt):
        """PE transposes of x (fp32r); PSUM->SBUF copies on ScalarE."""
        xT = xtpool.tile([P, DIN], F32R, tag="xT", name=f"xT{bt}")
        for ic in range(KC):
            tps = ps_t.tile([P, P], F32R, tag="tps", name="tps")
            nc.tensor.transpose(tps[:], xt[:, ic * P:(ic + 1) * P], identr[:])
            nc.scalar.copy(xT[:, ic * P:(ic + 1) * P], tps[:])
        return xT

    def issue_gt(gsrc, bt):
        gps = ps_t.tile([G, P], F32, tag="tps", name="gps")
        nc.tensor.transpose(gps[:], gsrc, ident32[:])
        gT = gtpool.tile([G, P], F32R, tag="gT", name=f"gT{bt}")
        nc.scalar.copy(gT[:], gps[:])
        return gT

    def matmul_y(xT, j):
        y = ps_y.tile([P, DOUT], F32, tag="y", name="y")
        for ic in range(KC):
            nc.tensor.matmul(
                y[:],
                xT[:, ic * P:(ic + 1) * P],
                w_sb[:, (j * KC + ic) * DOUT:(j * KC + ic + 1) * DOUT],
                start=(ic == 0),
                stop=(ic == KC - 1),
            )
        return y

    def fused_step(acc, y, gcol, seed=None):
        # acc = y * g[:,j] + (yb at j=0 else acc): one VectorE op drains the
        # y PSUM bank, applies the group weight, and accumulates.
        nc.vector.scalar_tensor_tensor(
            acc[:], y[:], gcol, (seed if seed is not None else acc)[:], MULT, ADD
        )

    # ---- phase A: tiles 0..7, group loop outermost (paced by W arrival) ----
    # All transposes run as one block BEFORE the sweeps, while the HAM clock
    # is still cold: their dependency gaps are harmless at 4/8, and the
    # sweeps that follow are a dense 100%-duty matmul stream that keeps the
    # released 8/8 clock from re-throttling.
    trs = {}
    gts = {}
    for k in range(PB):
        trs[k] = issue_transpose_x(xts[k], k)
        gts[k] = issue_gt(gA[:, k, :], k)

    def transpose_chunk(xt, xT, ic):
        tps = ps_t.tile([P, P], F32R, tag="tps", name="tps")
        nc.tensor.transpose(tps[:], xt[:, ic * P:(ic + 1) * P], identr[:])
        nc.scalar.copy(xT[:, ic * P:(ic + 1) * P], tps[:])

    # phase-B transposes are prefetched one tile ahead, interleaved between
    # the previous tile's matmul groups: each transpose's PSUM->SBUF copy
    # drains under dense matmul cover, so no PE hole forms at tile starts.
    def prefetch_phase_b(bt, j):
        if not (PB <= bt < NBT) or j not in (2, 3, 4, 5, 6):
            return
        if j == 2:
            trs[bt] = xtpool.tile([P, DIN], F32R, tag="xT", name=f"xT{bt}")
        if j < 6:
            transpose_chunk(xts[bt], trs[bt], j - 2)
        else:
            gts[bt] = issue_gt(gB[:, bt - PB, :], bt)

    accs = {}
    for j in range(G):
        for k in range(PB):
            gcol = gA[:, k, j:j + 1]
            if j == 0:
                yb = ps_yb.tile([P, DOUT], F32, tag="yb", name=f"yb{k}")
                nc.tensor.matmul(yb[:], gts[k][:], bias_sb[:], start=True, stop=True)
                y = matmul_y(trs[k], j)
                # the fused op may read only one PSUM operand; stage the
                # bias term through SBUF on the (otherwise idle) ScalarE
                ybs = ybspool.tile([P, DOUT], F32, tag="ybs", name=f"ybs{k}")
                nc.scalar.copy(ybs[:], yb[:])
                acc = accpool.tile([P, DOUT], F32, tag="acc", name=f"acc{k}")
                accs[k] = acc
                fused_step(acc, y, gcol, seed=ybs)
            else:
                y = matmul_y(trs[k], j)
                fused_step(accs[k], y, gcol)
            if k == PB - 1:
                # prefetch the first phase-B tile during sweeps j=2..6
                prefetch_phase_b(PB, j)

    for k in range(PB):
        nc.sync.dma_start(out_ap[k * P:(k + 1) * P, :], accs[k][:])

    # ---- phase B: tiles 8..15, tile loop outermost (W fully resident) ----
    for bt in range(PB, NBT):
        k = bt - PB
        yb = ps_yb.tile([P, DOUT], F32, tag="yb", name=f"yb{bt}")
        nc.tensor.matmul(yb[:], gts[bt][:], bias_sb[:], start=True, stop=True)
        ybs = ybspool.tile([P, DOUT], F32, tag="ybs", name=f"ybs{bt}")
        nc.scalar.copy(ybs[:], yb[:])
        acc = accpool.tile([P, DOUT], F32, tag="acc", name=f"acc{bt}")
        for j in range(G):
            y = matmul_y(trs[bt], j)
            gcol = gB[:, k, j:j + 1]
            fused_step(acc, y, gcol, seed=ybs if j == 0 else None)
            prefetch_phase_b(bt + 1, j)
        nc.sync.dma_start(out_ap[bt * P:(bt + 1) * P, :], acc[:])


def _build():
    nc = bacc.Bacc("TRN2", target_bir_lowering=False, debug=False)
    # x/weight/bias declared float32r (same 4-byte layout as fp32 on the
    # host) so DMA feeds the FP32r matmuls/transposes with no conversion
    x_ap = nc.dram_tensor("x", [BC, DIN], F32R, kind="ExternalInput").ap()
    g_ap = nc.dram_tensor("g", [BC, G], F32, kind="ExternalInput").ap()
    w_ap = nc.dram_tensor("weight", [G, DIN, DOUT], F32R, kind="ExternalInput").ap()
    bias_ap = nc.dram_tensor("bias_p", [G, DOUT], F32R, kind="ExternalInput").ap()
    out_ap = nc.dram_tensor("out", [BC, DOUT], F32, kind="ExternalOutput").ap()

    from contextlib import ExitStack

    with tile.TileContext(nc) as tc:
        with ExitStack() as ctx:
            _emit(nc, tc, out_ap, x_ap, g_ap, w_ap, bias_ap, ctx)
    nc.compile()
    return nc


_NC = None
last_result = None


def kernel(x, g, weight, bias_p):
    global _NC, last_result
    if _NC is None:
        _NC = _build()

    x = np.ascontiguousarray(np.asarray(x, dtype=np.float32))
    g = np.ascontiguousarray(np.asarray(g, dtype=np.float32))
    weight = np.ascontiguousarray(np.asarray(weight, dtype=np.float32))
    bias_p = np.ascontiguousarray(np.asarray(bias_p, dtype=np.float32))

    in_maps = [
        {
            "x": x[c * BC:(c + 1) * BC],
            "g": g[c * BC:(c + 1) * BC],
            "weight": weight,
            "bias_p": bias_p,
        }
        for c in range(NCORES)
    ]
    res = run_bass_kernel_spmd(_NC, in_maps, core_ids=list(range(NCORES)))
    last_result = res
    return np.concatenate([r["out"] for r in res.results], axis=0)


# revision 28
# speedup vs baseline: 1.0197x; 1.0084x over previous
"""GroupLinear (soft MoE routing) Trainium2 Bass kernel.

Computes out[b,o] = sum_j g[b,j] * (x[b,:] @ W[j,:,:])[o] + (g @ bias_p)[b,o]
for B=16384, G=16, DIN=DOUT=512, fp32.

Sharding: data-parallel over batch across 8 NeuronCores (2048 rows/core);
weight + bias replicated.

Per-core schedule (PE-roofline oriented; the PE stream is the critical path
at ~231 ns per 512-row fp32r matmul, and any multi-us PE stall also drops
the HAM clock to 4/8 for tens of us — so every engine that gates the PE
must stay far ahead):
  - W streams on the SP HWDGE queue; x0/g/bias startup loads on the
    Activation queue so W[0] and x0 transfer concurrently. g tiles for a
    whole phase load as ONE batched DMA ([128, 8, 16]).
  - fp32 warmup matmuls cover the framework preamble -> W[0] arrival window
    and start the HAM clock ramp.
  - group-mix accumulation uses the fused VectorE scalar_tensor_tensor:
    acc = y * g[:,j] + acc (one op per (j,tile) instead of scale+add, with
    the j=0 op seeding from the bias matmul: acc = y*g0 + yb). VectorE is
    the only PSUM drain for y (8.5us per 16-group chain vs 15.1us of PE per
    tile); ScalarE only does the small transpose copies, so neither can
    back-pressure the PE's PSUM rotation.
  - phase A (tiles 0-7): group loop outermost, paced by W arrival; x
    transposes (fp32r identity, 1.5 cy/row) + gT + bias matmul emitted
    per-tile inside the j=0 sweep.
  - phase B (tiles 8-15): tile loop outermost (W resident); each tile's
    output DMA streams out as soon as its chain ends - no tail burst.
"""

import numpy as np

import concourse.bass as bass
import concourse.tile as tile
from concourse import bacc, mybir
from concourse.bass_utils import run_bass_kernel_spmd
from concourse.masks import make_identity

B, G, DIN, DOUT = 16384, 16, 512, 512
NCORES = 8
BC = B // NCORES          # rows per core
P = 128                   # partitions
NBT = BC // P             # batch tiles per core (16)
KC = DIN // P             # contraction chunks (4)
PB = 8                    # batch tiles per phase
NPH = NBT // PB           # phases (2)

F32 = mybir.dt.float32
F32R = mybir.dt.float32r
MULT = mybir.AluOpType.mult
ADD = mybir.AluOpType.add


def _emit(nc, tc, out_ap, x_ap, g_ap, w_ap, bias_ap, ctx):
    const_pool = ctx.enter_context(tc.tile_pool(name="const", bufs=1))
    wpool = ctx.enter_context(tc.tile_pool(name="wpool", bufs=1))
    xpool = ctx.enter_context(tc.tile_pool(name="xpool", bufs=9))
    gpool = ctx.enter_context(tc.tile_pool(name="gpool", bufs=2))
    xtpool = ctx.enter_context(tc.tile_pool(name="xtpool", bufs=PB + 1))
    gtpool = ctx.enter_context(tc.tile_pool(name="gtpool", bufs=PB + 1))
    accpool = ctx.enter_context(tc.tile_pool(name="accpool", bufs=PB + 1))
    ybspool = ctx.enter_context(tc.tile_pool(name="ybspool", bufs=3))
    ps_y = ctx.enter_context(tc.tile_pool(name="ps_y", bufs=4, space="PSUM"))
    ps_yb = ctx.enter_context(tc.tile_pool(name="ps_yb", bufs=1, space="PSUM"))
    ps_t = ctx.enter_context(tc.tile_pool(name="ps_t", bufs=3, space="PSUM"))

    # fp32r identity: transpose cost is keyed on the moving operand (the
    # identity); fp32r streams 1.5 cy/row vs 2.0 for fp32. Built as fp32
    # (gpsimd memset/affine_select reject f32r), rounded into an f32r tile
    # via ScalarE copy (satisfies the rounded-to-FP32r BIR check; 0/1 are
    # exact). A plain fp32 identity serves the fp32 g transposes.
    ident32 = const_pool.tile([P, P], F32, name="ident32")
    make_identity(nc, ident32)
    identr = const_pool.tile([P, P], F32R, name="identr")
    nc.scalar.copy(identr[:], ident32[:])

    # PE warmup: dependency-free matmuls covering preamble -> W[0] arrival,
    # ramping the HAM clock before the real stream begins.
    dum = const_pool.tile([P, DOUT], F32, name="dum")
    nc.gpsimd.memset(dum[:], 1.0)
    for wi in range(3):
        wps = ps_t.tile([P, DOUT], F32, tag="tps", name="wps")
        nc.tensor.matmul(wps[:], dum[:, 0:P], dum[:], start=True, stop=True)

    w_sb = wpool.tile([P, G * KC * DOUT], F32R, name="w_sb")

    def issue_w(j):
        for ic in range(KC):
            nc.sync.dma_start(
                w_sb[:, (j * KC + ic) * DOUT:(j * KC + ic + 1) * DOUT],
                w_ap[j, ic * P:(ic + 1) * P, :],
            )

    def issue_x(bt):
        xt = xpool.tile([P, DIN], F32R, tag="xt", name=f"xt{bt}")
        nc.sync.dma_start(xt[:], x_ap[bt * P:(bt + 1) * P, :])
        return xt

    # EVERYTHING goes on the single SP HWDGE queue, strictly FIFO: under
    # load the DMA engines starve the other queues (Act-queue smalls seen
    # landing 9us late), so multi-queue issue makes arrival times
    # unpredictable. On one queue, arrival order == this order, and the
    # x-tile cadence (~0.6us apart) stays ahead of the j=0 sweep's
    # consumption (~1.6us/tile). Any >750ns PE hole after the HAM clock
    # releases trips a ~24us half-clock window, so arrivals must lead use.
    xts = {0: issue_x(0)}
    issue_w(0)
    gA = gpool.tile([P, PB, G], F32, tag="g", name="gA")
    nc.sync.dma_start(
        gA[:], g_ap[0:PB * P, :].rearrange("(k p) j -> p k j", p=P)
    )
    bias_sb = const_pool.tile([G, DOUT], F32R, name="bias_sb")
    nc.sync.dma_start(bias_sb[:], bias_ap[:, :])
    for bt in range(1, PB):
        xts[bt] = issue_x(bt)
    issue_w(1)
    issue_w(2)
    for bt in range(PB, NBT):
        xts[bt] = issue_x(bt)
    gB = gpool.tile([P, PB, G], F32, tag="g", name="gB")
    nc.sync.dma_start(
        gB[:], g_ap[PB * P:2 * PB * P, :].rearrange("(k p) j -> p k j", p=P)
    )
    for j in range(3, G):
        issue_w(j)

    def issue_transpose_x(xt, bt):
        """PE transposes of x (fp32r); PSUM->SBUF copies on ScalarE."""
        xT = xtpool.tile([P, DIN], F32R, tag="xT", name=f"xT{bt}")
        for ic in range(KC):
            tps = ps_t.tile([P, P], F32R, tag="tps", name="tps")
            nc.tensor.transpose(tps[:], xt[:, ic * P:(ic + 1) * P], identr[:])
            nc.scalar.copy(xT[:, ic * P:(ic + 1) * P], tps[:])
        return xT

    def issue_gt(gsrc, bt):
        gps = ps_t.tile([G, P], F32, tag="tps", name="gps")
        nc.tensor.transpose(gps[:], gsrc, ident32[:])
        gT = gtpool.tile([G, P], F32R, tag="gT", name=f"gT{bt}")
        nc.scalar.copy(gT[:], gps[:])
        return gT

    def matmul_y(xT, j):
        y = ps_y.tile([P, DOUT], F32, tag="y", name="y")
        for ic in range(KC):
            nc.tensor.matmul(
                y[:],
                xT[:, ic * P:(ic + 1) * P],
                w_sb[:, (j * KC + ic) * DOUT:(j * KC + ic + 1) * DOUT],
                start=(ic == 0),
                stop=(ic == KC - 1),
            )
        return y

    def fused_step(acc, y, gcol, seed=None):
        # acc = y * g[:,j] + (yb at j=0 else acc): one VectorE op drains the
        # y PSUM bank, applies the group weight, and accumulates.
        nc.vector.scalar_tensor_tensor(
            acc[:], y[:], gcol, (seed if seed is not None else acc)[:], MULT, ADD
        )

    # ---- phase A: tiles 0..7, group loop outermost (paced by W arrival) ----
    # j=0 sweep carries the per-tile prep software-pipelined one tile ahead
    # (tile k+1's transposes run between tile k's matmul groups, so their
    # PSUM->SBUF copies drain under matmul cover and no PE hole forms).
    trs = {}
    gts = {}
    trs[0] = issue_transpose_x(xts[0], 0)

    def transpose_chunk(xt, xT, ic):
        tps = ps_t.tile([P, P], F32R, tag="tps", name="tps")
        nc.tensor.transpose(tps[:], xt[:, ic * P:(ic + 1) * P], identr[:])
        nc.scalar.copy(xT[:, ic * P:(ic + 1) * P], tps[:])

    # phase-B transposes are prefetched one tile ahead, interleaved between
    # the previous tile's matmul groups: each transpose's PSUM->SBUF copy
    # drains under dense matmul cover, so no PE hole forms at tile starts.
    def prefetch_phase_b(bt, j):
        if not (PB <= bt < NBT) or j not in (2, 3, 4, 5, 6):
            return
        if j == 2:
            trs[bt] = xtpool.tile([P, DIN], F32R, tag="xT", name=f"xT{bt}")
        if j < 6:
            transpose_chunk(xts[bt], trs[bt], j - 2)
        else:
            gts[bt] = issue_gt(gB[:, bt - PB, :], bt)

    accs = {}
    for j in range(G):
        for k in range(PB):
            gcol = gA[:, k, j:j + 1]
            if j == 0:
                y = matmul_y(trs[k], j)
                if k + 1 < PB:
                    trs[k + 1] = issue_transpose_x(xts[k + 1], k + 1)
                gts[k] = issue_gt(gA[:, k, :], k)
                yb = ps_yb.tile([P, DOUT], F32, tag="yb", name=f"yb{k}")
                nc.tensor.matmul(yb[:], gts[k][:], bias_sb[:], start=True, stop=True)
                # the fused op may read only one PSUM operand; stage the
                # bias term through SBUF on the (otherwise idle) ScalarE
                ybs = ybspool.tile([P, DOUT], F32, tag="ybs", name=f"ybs{k}")
                nc.scalar.copy(ybs[:], yb[:])
                acc = accpool.tile([P, DOUT], F32, tag="acc", name=f"acc{k}")
                accs[k] = acc
                fused_step(acc, y, gcol, seed=ybs)
            else:
                y = matmul_y(trs[k], j)
                fused_step(accs[k], y, gcol)
            if k == PB - 1:
                # prefetch the first phase-B tile during sweeps j=2..6
                prefetch_phase_b(PB, j)

    for k in range(PB):
        nc.sync.dma_start(out_ap[k * P:(k + 1) * P, :], accs[k][:])

    # ---- phase B: tiles 8..15, tile loop outermost (W fully resident) ----
    for bt in range(PB, NBT):
        k = bt - PB
        yb = ps_yb.tile([P, DOUT], F32, tag="yb", name=f"yb{bt}")
        nc.tensor.matmul(yb[:], gts[bt][:], bias_sb[:], start=True, stop=True)
        ybs = ybspool.tile([P, DOUT], F32, tag="ybs", name=f"ybs{bt}")
        nc.scalar.copy(ybs[:], yb[:])
        acc = accpool.tile([P, DOUT], F32, tag="acc", name=f"acc{bt}")
        for j in range(G):
            y = matmul_y(trs[bt], j)
            gcol = gB[:, k, j:j + 1]
            fused_step(acc, y, gcol, seed=ybs if j == 0 else None)
            prefetch_phase_b(bt + 1, j)
        nc.sync.dma_start(out_ap[bt * P:(bt + 1) * P, :], acc[:])


def _build():
    nc = bacc.Bacc("TRN2", target_bir_lowering=False, debug=False)
    # x/weight/bias declared float32r (same 4-byte layout as fp32 on the
    # host) so DMA feeds the FP32r matmuls/transposes with no conversion
    x_ap = nc.dram_tensor("x", [BC, DIN], F32R, kind="ExternalInput").ap()
    g_ap = nc.dram_tensor("g", [BC, G], F32, kind="ExternalInput").ap()
    w_ap = nc.dram_tensor("weight", [G, DIN, DOUT], F32R, kind="ExternalInput").ap()
    bias_ap = nc.dram_tensor("bias_p", [G, DOUT], F32R, kind="ExternalInput").ap()
    out_ap = nc.dram_tensor("out", [BC, DOUT], F32, kind="ExternalOutput").ap()

    from contextlib import ExitStack

    with tile.TileContext(nc) as tc:
        with ExitStack() as ctx:
            _emit(nc, tc, out_ap, x_ap, g_ap, w_ap, bias_ap, ctx)
    nc.compile()
    return nc


_NC = None
last_result = None


def kernel(x, g, weight, bias_p):
    global _NC, last_result
    if _NC is None:
        _NC = _build()

    x = np.ascontiguousarray(np.asarray(x, dtype=np.float32))
    g = np.ascontiguousarray(np.asarray(g, dtype=np.float32))
    weight = np.ascontiguousarray(np.asarray(weight, dtype=np.float32))
    bias_p = np.ascontiguousarray(np.asarray(bias_p, dtype=np.float32))

    in_maps = [
        {
            "x": x[c * BC:(c + 1) * BC],
            "g": g[c * BC:(c + 1) * BC],
            "weight": weight,
            "bias_p": bias_p,
        }
        for c in range(NCORES)
    ]
    res = run_bass_kernel_spmd(_NC, in_maps, core_ids=list(range(NCORES)))
    last_result = res
    return np.concatenate([r["out"] for r in res.results], axis=0)


# revision 30
# speedup vs baseline: 1.0715x; 1.0508x over previous
"""GroupLinear (soft MoE routing) Trainium2 Bass kernel.

Computes out[b,o] = sum_j g[b,j] * (x[b,:] @ W[j,:,:])[o] + (g @ bias_p)[b,o]
for B=16384, G=16, DIN=DOUT=512, fp32.

Sharding: data-parallel over batch across 8 NeuronCores (2048 rows/core);
weight + bias replicated.

Per-core schedule (PE-roofline oriented; the PE stream is the critical path
at ~231 ns per 512-row fp32r matmul, and any multi-us PE stall also drops
the HAM clock to 4/8 for tens of us — so every engine that gates the PE
must stay far ahead):
  - W streams on the SP HWDGE queue; x0/g/bias startup loads on the
    Activation queue so W[0] and x0 transfer concurrently. g tiles for a
    whole phase load as ONE batched DMA ([128, 8, 16]).
  - fp32 warmup matmuls cover the framework preamble -> W[0] arrival window
    and start the HAM clock ramp.
  - group-mix accumulation uses the fused VectorE scalar_tensor_tensor:
    acc = y * g[:,j] + acc (one op per (j,tile) instead of scale+add, with
    the j=0 op seeding from the bias matmul: acc = y*g0 + yb). VectorE is
    the only PSUM drain for y (8.5us per 16-group chain vs 15.1us of PE per
    tile); ScalarE only does the small transpose copies, so neither can
    back-pressure the PE's PSUM rotation.
  - phase A (tiles 0-7): group loop outermost, paced by W arrival; x
    transposes (fp32r identity, 1.5 cy/row) + gT + bias matmul emitted
    per-tile inside the j=0 sweep.
  - phase B (tiles 8-15): tile loop outermost (W resident); each tile's
    output DMA streams out as soon as its chain ends - no tail burst.
"""

import numpy as np

import concourse.bass as bass
import concourse.tile as tile
from concourse import bacc, mybir
from concourse.bass_utils import run_bass_kernel_spmd
from concourse.masks import make_identity

B, G, DIN, DOUT = 16384, 16, 512, 512
NCORES = 8
BC = B // NCORES          # rows per core
P = 128                   # partitions
NBT = BC // P             # batch tiles per core (16)
KC = DIN // P             # contraction chunks (4)
PB = 8                    # batch tiles per phase
NPH = NBT // PB           # phases (2)

F32 = mybir.dt.float32
F32R = mybir.dt.float32r
MULT = mybir.AluOpType.mult
ADD = mybir.AluOpType.add


def _emit(nc, tc, out_ap, x_ap, g_ap, w_ap, bias_ap, ctx):
    const_pool = ctx.enter_context(tc.tile_pool(name="const", bufs=1))
    wpool = ctx.enter_context(tc.tile_pool(name="wpool", bufs=1))
    xpool = ctx.enter_context(tc.tile_pool(name="xpool", bufs=9))
    gpool = ctx.enter_context(tc.tile_pool(name="gpool", bufs=2))
    xtpool = ctx.enter_context(tc.tile_pool(name="xtpool", bufs=PB + 1))
    gtpool = ctx.enter_context(tc.tile_pool(name="gtpool", bufs=PB + 1))
    accpool = ctx.enter_context(tc.tile_pool(name="accpool", bufs=PB + 1))
    ybspool = ctx.enter_context(tc.tile_pool(name="ybspool", bufs=3))
    ps_y = ctx.enter_context(tc.tile_pool(name="ps_y", bufs=4, space="PSUM"))
    ps_yb = ctx.enter_context(tc.tile_pool(name="ps_yb", bufs=1, space="PSUM"))
    ps_t = ctx.enter_context(tc.tile_pool(name="ps_t", bufs=3, space="PSUM"))

    # fp32r identity: transpose cost is keyed on the moving operand (the
    # identity); fp32r streams 1.5 cy/row vs 2.0 for fp32. Built as fp32
    # (gpsimd memset/affine_select reject f32r), rounded into an f32r tile
    # via ScalarE copy (satisfies the rounded-to-FP32r BIR check; 0/1 are
    # exact). A plain fp32 identity serves the fp32 g transposes.
    ident32 = const_pool.tile([P, P], F32, name="ident32")
    make_identity(nc, ident32)
    identr = const_pool.tile([P, P], F32R, name="identr")
    nc.scalar.copy(identr[:], ident32[:])

    # PE warmup: dependency-free matmuls covering preamble -> W[0] arrival,
    # ramping the HAM clock before the real stream begins.
    # 5 warmups (~1.06us each cold) stretch PE cover until the x tiles'
    # DMA supply (0.65us cadence + 2.4us issue/transfer/sem latency) gets
    # ahead of the j=0 sweep's demand.
    dum = const_pool.tile([P, DOUT], F32, name="dum")
    nc.gpsimd.memset(dum[:], 1.0)
    for wi in range(5):
        wps = ps_t.tile([P, DOUT], F32, tag="tps", name="wps")
        nc.tensor.matmul(wps[:], dum[:, 0:P], dum[:], start=True, stop=True)

    w_sb = wpool.tile([P, G * KC * DOUT], F32R, name="w_sb")

    def issue_w(j):
        for ic in range(KC):
            nc.sync.dma_start(
                w_sb[:, (j * KC + ic) * DOUT:(j * KC + ic + 1) * DOUT],
                w_ap[j, ic * P:(ic + 1) * P, :],
            )

    def issue_x(bt):
        xt = xpool.tile([P, DIN], F32R, tag="xt", name=f"xt{bt}")
        nc.sync.dma_start(xt[:], x_ap[bt * P:(bt + 1) * P, :])
        return xt

    # EVERYTHING goes on the single SP HWDGE queue, strictly FIFO: under
    # load the DMA engines starve the other queues (Act-queue smalls seen
    # landing 9us late), so multi-queue issue makes arrival times
    # unpredictable. On one queue, arrival order == this order, and the
    # x-tile cadence (~0.6us apart) stays ahead of the j=0 sweep's
    # consumption (~1.6us/tile). Any >750ns PE hole after the HAM clock
    # releases trips a ~24us half-clock window, so arrivals must lead use.
    xts = {0: issue_x(0)}
    issue_w(0)
    xts[1] = issue_x(1)
    gA = gpool.tile([P, PB, G], F32, tag="g", name="gA")
    nc.sync.dma_start(
        gA[:], g_ap[0:PB * P, :].rearrange("(k p) j -> p k j", p=P)
    )
    bias_sb = const_pool.tile([G, DOUT], F32R, name="bias_sb")
    nc.sync.dma_start(bias_sb[:], bias_ap[:, :])
    for bt in range(2, PB):
        xts[bt] = issue_x(bt)
    issue_w(1)
    issue_w(2)
    for bt in range(PB, NBT):
        xts[bt] = issue_x(bt)
    gB = gpool.tile([P, PB, G], F32, tag="g", name="gB")
    nc.sync.dma_start(
        gB[:], g_ap[PB * P:2 * PB * P, :].rearrange("(k p) j -> p k j", p=P)
    )
    for j in range(3, G):
        issue_w(j)

    def issue_transpose_x(xt, bt):
        """PE transposes of x (fp32r); PSUM->SBUF copies on ScalarE."""
        xT = xtpool.tile([P, DIN], F32R, tag="xT", name=f"xT{bt}")
        for ic in range(KC):
            tps = ps_t.tile([P, P], F32R, tag="tps", name="tps")
            nc.tensor.transpose(tps[:], xt[:, ic * P:(ic + 1) * P], identr[:])
            nc.scalar.copy(xT[:, ic * P:(ic + 1) * P], tps[:])
        return xT

    def issue_gt(gsrc, bt):
        gps = ps_t.tile([G, P], F32, tag="tps", name="gps")
        nc.tensor.transpose(gps[:], gsrc, ident32[:])
        gT = gtpool.tile([G, P], F32R, tag="gT", name=f"gT{bt}")
        nc.scalar.copy(gT[:], gps[:])
        return gT

    def matmul_y(xT, j):
        y = ps_y.tile([P, DOUT], F32, tag="y", name="y")
        for ic in range(KC):
            nc.tensor.matmul(
                y[:],
                xT[:, ic * P:(ic + 1) * P],
                w_sb[:, (j * KC + ic) * DOUT:(j * KC + ic + 1) * DOUT],
                start=(ic == 0),
                stop=(ic == KC - 1),
            )
        return y

    def fused_step(acc, y, gcol, seed=None):
        # acc = y * g[:,j] + (yb at j=0 else acc): one VectorE op drains the
        # y PSUM bank, applies the group weight, and accumulates.
        nc.vector.scalar_tensor_tensor(
            acc[:], y[:], gcol, (seed if seed is not None else acc)[:], MULT, ADD
        )

    # ---- phase A: tiles 0..7, group loop outermost (paced by W arrival) ----
    # j=0 sweep carries the per-tile prep software-pipelined one tile ahead
    # (tile k+1's transposes run between tile k's matmul groups, so their
    # PSUM->SBUF copies drain under matmul cover and no PE hole forms).
    trs = {}
    gts = {}
    trs[0] = issue_transpose_x(xts[0], 0)

    def transpose_chunk(xt, xT, ic):
        tps = ps_t.tile([P, P], F32R, tag="tps", name="tps")
        nc.tensor.transpose(tps[:], xt[:, ic * P:(ic + 1) * P], identr[:])
        nc.scalar.copy(xT[:, ic * P:(ic + 1) * P], tps[:])

    # phase-B transposes are prefetched one tile ahead, interleaved between
    # the previous tile's matmul groups: each transpose's PSUM->SBUF copy
    # drains under dense matmul cover, so no PE hole forms at tile starts.
    def prefetch_phase_b(bt, j):
        if not (PB <= bt < NBT) or j not in (2, 3, 4, 5, 6):
            return
        if j == 2:
            trs[bt] = xtpool.tile([P, DIN], F32R, tag="xT", name=f"xT{bt}")
        if j < 6:
            transpose_chunk(xts[bt], trs[bt], j - 2)
        else:
            gts[bt] = issue_gt(gB[:, bt - PB, :], bt)

    accs = {}
    for j in range(G):
        for k in range(PB):
            gcol = gA[:, k, j:j + 1]
            if j == 0:
                y = matmul_y(trs[k], j)
                if k + 1 < PB:
                    trs[k + 1] = issue_transpose_x(xts[k + 1], k + 1)
                gts[k] = issue_gt(gA[:, k, :], k)
                yb = ps_yb.tile([P, DOUT], F32, tag="yb", name=f"yb{k}")
                nc.tensor.matmul(yb[:], gts[k][:], bias_sb[:], start=True, stop=True)
                # the fused op may read only one PSUM operand; stage the
                # bias term through SBUF on the (otherwise idle) ScalarE
                ybs = ybspool.tile([P, DOUT], F32, tag="ybs", name=f"ybs{k}")
                nc.scalar.copy(ybs[:], yb[:])
                acc = accpool.tile([P, DOUT], F32, tag="acc", name=f"acc{k}")
                accs[k] = acc
                fused_step(acc, y, gcol, seed=ybs)
            else:
                y = matmul_y(trs[k], j)
                fused_step(accs[k], y, gcol)
            if k == PB - 1:
                # prefetch the first phase-B tile during sweeps j=2..6
                prefetch_phase_b(PB, j)

    for k in range(PB):
        nc.sync.dma_start(out_ap[k * P:(k + 1) * P, :], accs[k][:])

    # ---- phase B: tiles 8..15, tile loop outermost (W fully resident) ----
    for bt in range(PB, NBT):
        k = bt - PB
        yb = ps_yb.tile([P, DOUT], F32, tag="yb", name=f"yb{bt}")
        nc.tensor.matmul(yb[:], gts[bt][:], bias_sb[:], start=True, stop=True)
        ybs = ybspool.tile([P, DOUT], F32, tag="ybs", name=f"ybs{bt}")
        nc.scalar.copy(ybs[:], yb[:])
        acc = accpool.tile([P, DOUT], F32, tag="acc", name=f"acc{bt}")
        for j in range(G):
            y = matmul_y(trs[bt], j)
            gcol = gB[:, k, j:j + 1]
            fused_step(acc, y, gcol, seed=ybs if j == 0 else None)
            prefetch_phase_b(bt + 1, j)
        nc.sync.dma_start(out_ap[bt * P:(bt + 1) * P, :], acc[:])


def _build():
    nc = bacc.Bacc("TRN2", target_bir_lowering=False, debug=False)
    # x/weight/bias declared float32r (same 4-byte layout as fp32 on the
    # host) so DMA feeds the FP32r matmuls/transposes with no conversion
    x_ap = nc.dram_tensor("x", [BC, DIN], F32R, kind="ExternalInput").ap()
    g_ap = nc.dram_tensor("g", [BC, G], F32, kind="ExternalInput").ap()
    w_ap = nc.dram_tensor("weight", [G, DIN, DOUT], F32R, kind="ExternalInput").ap()
    bias_ap = nc.dram_tensor("bias_p", [G, DOUT], F32R, kind="ExternalInput").ap()
    out_ap = nc.dram_tensor("out", [BC, DOUT], F32, kind="ExternalOutput").ap()

    from contextlib import ExitStack

    with tile.TileContext(nc) as tc:
        with ExitStack() as ctx:
            _emit(nc, tc, out_ap, x_ap, g_ap, w_ap, bias_ap, ctx)
    nc.compile()
    return nc


_NC = None
last_result = None


def kernel(x, g, weight, bias_p):
    global _NC, last_result
    if _NC is None:
        _NC = _build()

    x = np.ascontiguousarray(np.asarray(x, dtype=np.float32))
    g = np.ascontiguousarray(np.asarray(g, dtype=np.float32))
    weight = np.ascontiguousarray(np.asarray(weight, dtype=np.float32))
    bias_p = np.ascontiguousarray(np.asarray(bias_p, dtype=np.float32))

    in_maps = [
        {
            "x": x[c * BC:(c + 1) * BC],
            "g": g[c * BC:(c + 1) * BC],
            "weight": weight,
            "bias_p": bias_p,
        }
        for c in range(NCORES)
    ]
    res = run_bass_kernel_spmd(_NC, in_maps, core_ids=list(range(NCORES)))
    last_result = res
    return np.concatenate([r["out"] for r in res.results], axis=0)
